# revision 3
# baseline (speedup 1.0000x reference)
"""GAT layer (single head) on 8 Trainium2 NeuronCores — optimized v2.

Layout (per core, dst-sharded, edges sorted by dst):
  - Table rows [h bf16(128) | one | el_hi | el_lo | garbage] = 512B, indexed
    by (quarter-window, row) so gather indices fit int16: window j holds the
    j-th local-tile-quarter of every rank: rows = rank*(QT_j*128) + local_off.
  - Phase 1 (bf16): project node slice per tile-group with batched DMA,
    write rows into hext_local; after each quarter completes, AllGather that
    quarter into its own shared window tensor (overlaps later phases).
  - Phase 2 per group: 4 dma_gathers (one per window); per tile: batched
    one-hot-transpose build (DVE/Pool) -> er via tiny PE matmuls, exv on Act
    (Prelu/Exp, all funcs in one act table set), per-chunk fused scaled
    one-hot (DVE) -> U|s PSUM matmul accumulate; epilogue on Act/DVE/PE with
    sigmoid computed as exp + reciprocal to avoid act-table swaps.
"""
import sys
sys.path.insert(0, "/opt/trn_rl_repo")
import numpy as np
import ml_dtypes

import concourse.bass as bass
import concourse.bacc as bacc
import concourse.mybir as mybir
import concourse.tile as tile
from concourse.masks import make_identity

bf16 = mybir.dt.bfloat16
f32 = mybir.dt.float32
P = 128
NCORES = 8
GT = 6          # node tiles per gather group (phase 2)
GP1 = 5         # tiles per phase-1 group
ES = 256        # table row elements (bf16) = 512B
COL_ONE = 128
COL_ELH = 129
COL_ELL = 130
NQ = 4          # quarter windows


def preprocess(in_feat, W, attn_l, attn_r, bias, fc_w, fc_b, src, dst):
    N, F = in_feat.shape
    D = W.shape[1]
    E = src.shape[0]
    assert N % NCORES == 0
    NLOC = N // NCORES
    T = (NLOC + P - 1) // P
    NPAD = T * P

    QROWS = 2 * NPAD
    order = np.argsort(dst, kind="stable")
    src_s = src[order].astype(np.int64)
    dst_s = dst[order].astype(np.int64)
    trow = (src_s // NLOC) * NPAD + (src_s % NLOC)
    q_of = trow // QROWS
    r_in_q = trow % QROWS
    core_bounds = np.searchsorted(dst_s, np.arange(NCORES + 1) * NLOC)

    counts = np.zeros((NCORES, T, NQ), np.int64)
    for c in range(NCORES):
        lo, hi = core_bounds[c], core_bounds[c + 1]
        t_loc = (dst_s[lo:hi] - c * NLOC) // P
        np.add.at(counts[c], (t_loc, q_of[lo:hi]), 1)
    K_tq = -(-counts.max(axis=0) // P)              # [T, NQ] chunks
    n_groups = -(-T // GT)
    sec_base = np.zeros((n_groups, NQ), np.int64)
    sec_len = np.zeros((n_groups, NQ), np.int64)
    slot_of = np.zeros((T, NQ), np.int64)
    acc = 0
    for g in range(n_groups):
        tlo, thi = g * GT, min((g + 1) * GT, T)
        for q in range(NQ):
            sec_base[g, q] = acc
            for t in range(tlo, thi):
                slot_of[t, q] = acc
                acc += K_tq[t, q]
            sec_len[g, q] = acc - sec_base[g, q]
    K_total = int(acc)
    KMAX = int(K_tq.sum(axis=1).max())

    params = dict(N=N, F=F, D=D, E=E, NLOC=NLOC, T=T, NPAD=NPAD,
                  QROWS=QROWS,
                  NQ=NQ, K_tq=K_tq, K_total=K_total, KMAX=KMAX,
                  n_groups=n_groups, sec_base=sec_base, sec_len=sec_len,
                  slot_of=slot_of)

    attn_l = np.asarray(attn_l, np.float32).reshape(-1)
    attn_r = np.asarray(attn_r, np.float32).reshape(-1)
    W = np.asarray(W, np.float32)
    Wext = np.concatenate([W, (W @ attn_l)[:, None], (W @ attn_r)[:, None],
                           np.zeros((F, 2), np.float32)], axis=1)  # [F, D+4]
    Wext_bf = Wext.astype(ml_dtypes.bfloat16)
    iota_row = np.tile(np.arange(P, dtype=ml_dtypes.bfloat16)[None, :], (P, 1))
    iota_col = np.arange(P, dtype=ml_dtypes.bfloat16)[:, None]
    bias_col = np.asarray(bias, np.float32).reshape(-1)[:, None]
    fc_w_bf = np.asarray(fc_w, ml_dtypes.bfloat16)
    nfc_b2 = -np.asarray(fc_b, np.float32).reshape(-1)[:, None]

    in_maps = []
    for c in range(NCORES):
        lo, hi = core_bounds[c], core_bounds[c + 1]
        t_loc = (dst_s[lo:hi] - c * NLOC) // P
        dr = ((dst_s[lo:hi] - c * NLOC) % P).astype(np.float32)
        qe = q_of[lo:hi]
        re = r_in_q[lo:hi]
        gid = t_loc * NQ + qe
        eo = np.lexsort((qe, t_loc))
        gid, dr, re, t_loc2, qe2 = gid[eo], dr[eo], re[eo], t_loc[eo], qe[eo]
        ne = len(gid)
        if ne:
            starts = np.r_[0, np.flatnonzero(np.diff(gid)) + 1]
            run_id = np.zeros(ne, np.int64)
            run_id[starts[1:]] = 1
            run_id = np.cumsum(run_id)
            pos = np.arange(ne) - starts[run_id]
        else:
            pos = np.zeros(0, np.int64)
        slot = slot_of[t_loc2, qe2] + pos // P
        prt = pos % P
        idx_flat = np.zeros(K_total * P, np.int64)   # pad -> row 0 (killed by dst_rel=-1)
        idx_flat[slot * P + prt] = re
        dst_rel = np.full((P, K_total), -1.0, np.float32)
        dst_rel[prt, slot] = dr
        n_idx = K_total * P
        wrapped = np.zeros((16, n_idx // 16), np.int16)
        ii = np.arange(n_idx)
        wrapped[ii % 16, ii // 16] = idx_flat.astype(np.int16)
        idx_all = np.tile(wrapped, (8, 1))

        in_featT = np.zeros((F, NPAD), ml_dtypes.bfloat16)
        in_featT[:, :NLOC] = np.asarray(in_feat, np.float32)[
            c * NLOC:(c + 1) * NLOC].T.astype(ml_dtypes.bfloat16)

        in_maps.append({
            "in_featT": in_featT,
            "Wext": Wext_bf,
            "fc_w": fc_w_bf,
            "nfc_b2": nfc_b2,
            "bias_col": bias_col,
            "iota_row": iota_row,
            "iota_col": iota_col,
            "dst_rel": dst_rel,
            "idx_all": idx_all,
        })
    return params, in_maps


def build(params, repeat=1):
    p = params
    T, NPAD = p["T"], p["NPAD"]
    QROWS = p["QROWS"]
    K_tq, K_total, KMAX = p["K_tq"], p["K_total"], p["KMAX"]
    n_groups, sec_base, sec_len, slot_of = (
        p["n_groups"], p["sec_base"], p["sec_len"], p["slot_of"])
    F, D = p["F"], p["D"]
    C = 2
    VTOT = NCORES * NPAD

    nc = bacc.Bacc("TRN2", target_bir_lowering=False, debug=False,
                   num_swdge_queues=4)
    in_featT = nc.dram_tensor("in_featT", [F, NPAD], bf16, kind="ExternalInput")
    Wext_d = nc.dram_tensor("Wext", [F, D + 4], bf16, kind="ExternalInput")
    fc_w_d = nc.dram_tensor("fc_w", [D, C], bf16, kind="ExternalInput")
    nfc_b2_d = nc.dram_tensor("nfc_b2", [C, 1], f32, kind="ExternalInput")
    bias_col_d = nc.dram_tensor("bias_col", [D, 1], f32, kind="ExternalInput")
    iota_row_d = nc.dram_tensor("iota_row", [P, P], bf16, kind="ExternalInput")
    iota_col_d = nc.dram_tensor("iota_col", [P, 1], bf16, kind="ExternalInput")
    dst_rel_d = nc.dram_tensor("dst_rel", [P, K_total], f32, kind="ExternalInput")
    idx_all_d = nc.dram_tensor("idx_all", [P, (K_total * P) // 16],
                               mybir.dt.int16, kind="ExternalInput")
    out2_d = nc.dram_tensor("out2", [C, NPAD], f32, kind="ExternalOutput")

    with tile.TileContext(nc) as tc:
        with (tc.tile_pool(name="const", bufs=1) as constp,
              tc.tile_pool(name="dram", bufs=1, space="DRAM") as dramp):
            Wext_sb = constp.tile([P, (F // P) * (D + 4)], bf16)
            Wext3 = Wext_sb[:].rearrange("p (h d) -> p h d", d=D + 4)
            for h in range(F // P):
                nc.sync.dma_start(out=Wext3[:, h, :],
                                  in_=Wext_d[h * P:(h + 1) * P, :])
            iota_row = constp.tile([P, P], bf16)
            nc.sync.dma_start(out=iota_row[:], in_=iota_row_d[:, :])
            iota_col = constp.tile([P, 1], bf16)
            nc.sync.dma_start(out=iota_col[:], in_=iota_col_d[:, :])
            bias_col = constp.tile([D, 1], f32)
            nc.sync.dma_start(out=bias_col[:], in_=bias_col_d[:, :])
            fc_w_sb = constp.tile([D, C], bf16)
            nc.sync.dma_start(out=fc_w_sb[:], in_=fc_w_d[:, :])
            nfc_b2_sb = constp.tile([C, 1], f32)
            nc.sync.dma_start(out=nfc_b2_sb[:], in_=nfc_b2_d[:, :])
            dst_rel = constp.tile([P, K_total], f32)
            nc.sync.dma_start(out=dst_rel[:], in_=dst_rel_d[:, :])
            idx_all = constp.tile([P, (K_total * P) // 16], mybir.dt.int16)
            nc.sync.dma_start(out=idx_all[:], in_=idx_all_d[:, :])
            er_cols = constp.tile([P, T], f32)
            er_cols_bf = constp.tile([P, T], bf16)
            ident = constp.tile([P, P], f32)
            make_identity(nc, ident[:])

            p1_groups = []
            t = 0
            while t < T:
                nt = min(GP1, T - t)
                p1_groups.append((t, nt))
                t += nt

            for _rep in range(repeat):
                hext_local = dramp.tile([NPAD, ES], bf16, name=f"hl{_rep}")
                hext_full = dramp.tile([VTOT, ES], bf16, addr_space="Shared",
                                       name=f"hf{_rep}")
                # ---------------- Phase 1 ----------------
                with (tc.tile_pool(name="p1", bufs=2) as p1,
                      tc.tile_pool(name="p1ps", bufs=2, space="PSUM") as p1ps):
                    for (t0, nt) in p1_groups:
                        lh = p1.tile([P, (F // P) * nt * P], bf16, tag="lh")
                        lh3 = lh[:].rearrange("p (h n) -> p h n", n=nt * P)
                        for h in range(F // P):
                            nc.sync.dma_start(
                                out=lh3[:, h, :],
                                in_=in_featT[h * P:(h + 1) * P,
                                             t0 * P:(t0 + nt) * P])
                        rowg = p1.tile([P, nt * ES], bf16, tag="rowg")
                        for ti in range(nt):
                            t = t0 + ti
                            hps = p1ps.tile([P, D + 4], f32, tag="hps",
                                            space="PSUM")
                            for h in range(F // P):
                                nc.tensor.matmul(
                                    out=hps[:],
                                    lhsT=lh3[:, h, ti * P:(ti + 1) * P],
                                    rhs=Wext3[:, h, :],
                                    start=(h == 0), stop=(h == F // P - 1))
                            rb = ti * ES
                            nc.scalar.activation(
                                out=rowg[:, rb:rb + D], in_=hps[:, 0:D],
                                func=mybir.ActivationFunctionType.Copy)
                            nc.vector.memset(
                                rowg[:, rb + COL_ONE:rb + COL_ONE + 1], 1.0)
                            nc.vector.tensor_copy(
                                out=rowg[:, rb + COL_ELH:rb + COL_ELH + 1],
                                in_=hps[:, D:D + 1])
                            nc.vector.tensor_tensor(
                                out=rowg[:, rb + COL_ELL:rb + COL_ELL + 1],
                                in0=hps[:, D:D + 1],
                                in1=rowg[:, rb + COL_ELH:rb + COL_ELH + 1],
                                op=mybir.AluOpType.subtract)
                            nc.vector.memset(
                                rowg[:, rb + COL_ELL + 1:rb + ES], 0.0)
                            nc.vector.tensor_copy(out=er_cols[:, t:t + 1],
                                                  in_=hps[:, D + 1:D + 2])
                            nc.vector.tensor_copy(out=er_cols_bf[:, t:t + 1],
                                                  in_=hps[:, D + 1:D + 2])
                        nc.sync.dma_start(
                            out=hext_local[t0 * P:(t0 + nt) * P, :]
                                .rearrange("(t p) d -> p t d", p=P),
                            in_=rowg[:].rearrange("p (t d) -> p t d", d=ES))

                # ---------------- AllGather ----------------
                nc.gpsimd.collective_compute(
                    "AllGather", mybir.AluOpType.bypass,
                    ins=[hext_local[:]],
                    outs=[hext_full[:]],
                    replica_groups=[list(range(NCORES))],
                )

                # ---------------- Phase 2 ----------------
                with (tc.tile_pool(name="p2", bufs=2) as p2,
                      tc.tile_pool(name="p2s", bufs=3) as p2s,
                      tc.tile_pool(name="p2oh", bufs=2) as p2oh,
                      tc.tile_pool(name="p2ps1", bufs=2, space="PSUM") as p2ps1,
                      tc.tile_pool(name="p2ps2", bufs=2, space="PSUM") as p2ps2,
                      tc.tile_pool(name="ups", bufs=2, space="PSUM") as ups):
                    for g in range(n_groups):
                        tlo, thi = g * GT, min((g + 1) * GT, T)
                        g_base = int(sec_base[g, 0])
                        g_len = int(sec_len[g].sum())
                        gt = p2.tile([P, g_len * ES], bf16, tag="gt")
                        gt3 = gt[:].rearrange("p (k d) -> p k d", d=ES)
                        for q in range(NQ):
                            sb, sl = int(sec_base[g, q]), int(sec_len[g, q])
                            if sl == 0:
                                continue
                            nidx = sl * P
                            nc.gpsimd.dma_gather(
                                gt3[:, sb - g_base:sb - g_base + sl, :],
                                hext_full[q * QROWS:(q + 1) * QROWS, :],
                                idx_all[:, (sb * P) // 16:
                                        (sb * P + nidx) // 16],
                                nidx, nidx, ES,
                                single_packet=False, queue_num=q % 4)
                        sigg = p2s.tile([C, (thi - tlo) * P], f32, tag="sigg")
                        # --- er[dst] per chunk, group-level accumulator ---
                        erp_g = p2s.tile([P, g_len], f32, tag="erp")
                        for t in range(tlo, thi):
                            K_t = int(K_tq[t].sum())
                            if K_t == 0:
                                continue
                            erT = p2ps2.tile([P, P], f32, tag="erT",
                                             space="PSUM")
                            nc.tensor.transpose(
                                out=erT[:],
                                in_=er_cols[:, t:t + 1].to_broadcast([P, P]),
                                identity=ident[:])
                            er_rep = p2oh.tile([P, P], bf16, tag="er_rep")
                            nc.scalar.activation(
                                out=er_rep[:], in_=erT[:],
                                func=mybir.ActivationFunctionType.Copy)
                            for q in range(NQ):
                                s0 = int(slot_of[t, q])
                                for s in range(s0, s0 + int(K_tq[t, q])):
                                    sc = p2oh.tile([P, P], bf16, tag="sc")
                                    nc.vector.scalar_tensor_tensor(
                                        out=sc[:], in0=iota_row[:],
                                        scalar=dst_rel[:, s:s + 1],
                                        in1=er_rep[:],
                                        op0=mybir.AluOpType.is_equal,
                                        op1=mybir.AluOpType.mult,
                                        accum_out=erp_g[:, s - g_base:
                                                        s - g_base + 1])
                        # --- exv = exp(lrelu(el + er)), group-level ---
                        asb_g = p2s.tile([P, g_len], f32, tag="asb")
                        for q in range(NQ):
                            sb, sl = int(sec_base[g, q]), int(sec_len[g, q])
                            if sl == 0:
                                continue
                            nc.vector.tensor_tensor(
                                out=asb_g[:, sb - g_base:sb - g_base + sl],
                                in0=gt3[:, sb - g_base:sb - g_base + sl,
                                        COL_ELH],
                                in1=gt3[:, sb - g_base:sb - g_base + sl,
                                        COL_ELL],
                                op=mybir.AluOpType.add)
                        tsb_g = p2s.tile([P, g_len], f32, tag="tsb")
                        nc.vector.tensor_tensor(
                            out=tsb_g[:], in0=asb_g[:], in1=erp_g[:],
                            op=mybir.AluOpType.add)
                        lrs_g = p2s.tile([P, g_len], f32, tag="lrs")
                        nc.scalar.activation(
                            out=lrs_g[:], in_=tsb_g[:],
                            func=mybir.ActivationFunctionType.Prelu,
                            alpha=0.2)
                        exv_g = p2s.tile([P, g_len], f32, tag="exv")
                        nc.scalar.activation(
                            out=exv_g[:], in_=lrs_g[:],
                            func=mybir.ActivationFunctionType.Exp)
                        for t in range(tlo, thi):
                            K_t = int(K_tq[t].sum())
                            if K_t == 0:
                                continue
                            slots = []
                            for q in range(NQ):
                                s0 = int(slot_of[t, q])
                                slots += list(range(s0, s0 + int(K_tq[t, q])))
                            # --- aggregate [U | s] ---
                            Ups = ups.tile([P, D + 1], f32, tag="Ups",
                                           space="PSUM")
                            for j, s in enumerate(slots):
                                selx = p2s.tile([P, P], bf16, tag="sx")
                                nc.vector.tensor_scalar(
                                    out=selx[:], in0=iota_row[:],
                                    scalar1=dst_rel[:, s:s + 1],
                                    scalar2=exv_g[:, s - g_base:s - g_base + 1],
                                    op0=mybir.AluOpType.is_equal,
                                    op1=mybir.AluOpType.mult)
                                nc.tensor.matmul(
                                    out=Ups[:], lhsT=selx[:],
                                    rhs=gt3[:, s - g_base, 0:D + 1],
                                    start=(j == 0), stop=(j == K_t - 1))
                            # --- epilogue ---
                            ssafe = p2s.tile([P, 1], f32, tag="ssafe")
                            nc.vector.tensor_scalar(
                                out=ssafe[:], in0=Ups[:, D:D + 1],
                                scalar1=1e-30, scalar2=None,
                                op0=mybir.AluOpType.max)
                            rs = p2s.tile([P, 1], f32, tag="rs")
                            nc.vector.reciprocal(out=rs[:], in_=ssafe[:])
                            t1 = p2s.tile([P, D], f32, tag="t1")
                            nc.scalar.activation(
                                out=t1[:], in_=Ups[:, 0:D],
                                func=mybir.ActivationFunctionType.Copy,
                                scale=rs[:])
                            aggT = p2ps1.tile([P, P], f32, tag="aggT",
                                              space="PSUM")
                            nc.tensor.transpose(out=aggT[:], in_=t1[:],
                                                identity=ident[:])
                            t2 = p2s.tile([D, P], bf16, tag="t2")
                            nc.scalar.activation(
                                out=t2[:], in_=aggT[:],
                                func=mybir.ActivationFunctionType.Relu,
                                bias=bias_col[:])
                            o2p = p2ps1.tile([C, P], f32, tag="o2p",
                                             space="PSUM")
                            nc.tensor.matmul(out=o2p[:], lhsT=fc_w_sb[:],
                                             rhs=t2[:], start=True, stop=True)
                            ze = p2s.tile([C, P], f32, tag="ze")
                            nc.scalar.activation(
                                out=ze[:], in_=o2p[:],
                                func=mybir.ActivationFunctionType.Exp,
                                scale=-1.0, bias=nfc_b2_sb[:])
                            zp = p2s.tile([C, P], f32, tag="zp")
                            nc.vector.tensor_scalar(
                                out=zp[:], in0=ze[:],
                                scalar1=1.0, scalar2=None,
                                op0=mybir.AluOpType.add)
                            nc.vector.reciprocal(
                                out=sigg[:, (t - tlo) * P:(t - tlo + 1) * P],
                                in_=zp[:])
                        nc.sync.dma_start(
                            out=out2_d[:, tlo * P:thi * P], in_=sigg[:])
    nc.finalize()
    return nc


def assemble(params, results):
    NLOC = params["NLOC"]
    outs = [results[c]["out2"][:, :NLOC].T for c in range(NCORES)]
    return np.concatenate(outs, axis=0).astype(np.float32)


from concourse.bass_utils import run_bass_kernel_spmd

_CACHE = {}


def kernel(in_feat, W, attn_l, attn_r, bias, fc_w, fc_b, src, dst):
    inputs = dict(in_feat=np.asarray(in_feat, np.float32),
                  W=np.asarray(W, np.float32),
                  attn_l=np.asarray(attn_l, np.float32),
                  attn_r=np.asarray(attn_r, np.float32),
                  bias=np.asarray(bias, np.float32),
                  fc_w=np.asarray(fc_w, np.float32),
                  fc_b=np.asarray(fc_b, np.float32),
                  src=np.asarray(src, np.int32),
                  dst=np.asarray(dst, np.int32))
    params, in_maps = preprocess(**inputs)
    key = (params["N"], params["F"], params["D"], params["E"],
           params["K_total"], tuple(params["K_tq"].reshape(-1).tolist()))
    if key not in _CACHE:
        _CACHE[key] = build(params)
    nc = _CACHE[key]
    res = run_bass_kernel_spmd(nc, in_maps, core_ids=list(range(NCORES)))
    return assemble(params, res.results)
